# revision 27
# baseline (speedup 1.0000x reference)
"""Bayesian GPLVM collapsed-ELBO kernel for Trainium2 (8 NeuronCores).

Sharding: data-parallel over n (2048 rows -> 256 per core). All O(n*q)
row prep (softplus, d1/d2, w1/w2, log-sums, KL pieces) is done on host
in float64 and shipped as small per-core bf16 operand blocks; the
device does only the O(n*m) / O(n*m^2) work:

  - psi2: for each of 17 ij-chunks (128 upper-triangle pairs each),
    exponent = zl_chunk^T @ p2 (33x128x256 bf16) on PE, Exp on ACT,
    free-axis n-sum on DVE (bf16 pair-sum fast path + f32 reduce); the
    last two chunks' n-sums ride their Exps via accum_out, and the
    exp(-s1[ij]) factor is folded in on the host after gathering.
  - psi1: exponent matmuls land in spare columns of the first psi2
    PSUM group so one ACTIVATE produces both; A += psi1^T y then
    accumulates in PSUM and drains via a DVE copy + DMA.

DMA plan: transfer rate scales with SBUF partition count and suffers
under ~1KB per-partition lines, so all 33-partition operands are
packed into TWO wide transfers (zl half A + p2 + p1z | zl half B + p2
dup) that land at partition bases 0 and 64 of one tile and ride two
different HWDGE queues in parallel (matmul lhsT/rhs must share a base
partition, hence the p2 duplicate). Only the Exp table is ever needed,
so there is a single ACT_TABLE_LOAD overlapping the input DMAs.

SBUF column maps (bf16):
  base-0 block  [33 x 1600]: 0:256 p2 | 256:1280 zl chunks 0-7 |
                             1280:1536 p1 (n-side) | 1536:1600 zs1
  base-64 block [33 x 1408]: 0:256 p2 | 256:1408 zl chunks 8-16
"""

import numpy as np

N, D, Q, M = 2048, 256, 16, 64
NCORES = 8
NLOC = N // NCORES          # 256
NPAIRS = 2080               # upper-triangle pairs of 64x64
NCHUNK = 17                 # ceil(2080 / 128)
# (start_chunk, n_chunks) per PSUM group; group 0 also carries the
# psi1 exponent, the last group's chunks are summed via accum_out
GROUPS = [(0, 3), (3, 4), (7, 4), (11, 4), (15, 2)]
HALF = 128

_compiled = None


def _build_bass():
    import concourse.bacc as bacc
    import concourse.bass as bass  # noqa: F401
    import concourse.mybir as mybir
    from concourse.tile import TileContext

    f32 = mybir.dt.float32
    bf16 = mybir.dt.bfloat16
    AF = mybir.ActivationFunctionType
    OP = mybir.AluOpType

    nc = bacc.Bacc("TRN2", target_bir_lowering=False, num_swdge_queues=2)

    za_d = nc.declare_dram_parameter("za", [33, 1600], bf16, isOutput=False)
    zb_d = nc.declare_dram_parameter("zb", [33, 1408], bf16, isOutput=False)
    y_d = nc.declare_dram_parameter("yb", [128, 512], bf16, isOutput=False)
    psi2_o = nc.declare_dram_parameter("out_psi2", [128, NCHUNK], f32,
                                       isOutput=True)
    a_o = nc.declare_dram_parameter("out_A", [M, D], f32, isOutput=True)

    with TileContext(nc) as tc:
        with (
            tc.tile_pool(name="const", bufs=1) as cpool,
            tc.tile_pool(name="scr", bufs=2) as spool,
            tc.tile_pool(name="psum0", bufs=1, space="PSUM") as ppool0,
            tc.tile_pool(name="psum", bufs=2, space="PSUM") as ppool,
            tc.tile_pool(name="psuma", bufs=1, space="PSUM") as ppool_a,
        ):
            zt = cpool.tile([97, 1600], bf16)
            yb = cpool.tile([128, 512], bf16)
            stats = cpool.tile([128, NCHUNK], f32)
            a_sb = cpool.tile([M, D], f32)

            # two partition-parallel packed input streams; the first
            # is split so the matmul train can start sooner
            nc.sync.dma_start(out=zt[0:33, 0:640], in_=za_d[:, 0:640])
            nc.sync.dma_start(out=zt[0:33, 640:1600], in_=za_d[:, 640:1600])
            nc.scalar.dma_start(out=zt[64:97, 0:1408], in_=zb_d[:, :])
            nc.scalar.dma_start(out=yb[:, :], in_=y_d[:, :])

            apsum = ppool_a.tile([M, D], f32)

            def zl_lhsT(ch):
                if ch < 8:
                    return zt[0:33, 256 + ch * 128:256 + (ch + 1) * 128]
                return zt[64:97, 256 + (ch - 8) * 128:
                          256 + (ch - 7) * 128]

            def p2_rhs(ch):
                return zt[0:33, 0:256] if ch < 8 else zt[64:97, 0:256]

            # psi2 exponent matmul train; the psi1 exponent rides in
            # group 0's spare PSUM columns (cols 768:896)
            ptiles = []
            for gi, (ch0, nch) in enumerate(GROUPS):
                pool = ppool0 if gi == 0 else ppool
                p2p = pool.tile([128, 4 * NLOC], f32,
                                tag="p2p" if gi else "p2p0")
                ptiles.append((p2p, ch0, nch))
                for j in range(nch):
                    ch = ch0 + j
                    nc.tensor.matmul(
                        p2p[:, j * NLOC:(j + 1) * NLOC],
                        lhsT=zl_lhsT(ch), rhs=p2_rhs(ch),
                        start=True, stop=True)
                if gi == 0:
                    for c in range(2):
                        nc.tensor.matmul(
                            p2p[:, 768 + c * 64:768 + (c + 1) * 64],
                            lhsT=zt[0:33, 1280 + c * 128:
                                    1280 + (c + 1) * 128],
                            rhs=zt[0:33, 1536:1600],
                            start=True, stop=True)

            exps = []
            for gi, (p2p, ch0, nch) in enumerate(ptiles):
                scr = spool.tile([128, 4 * NLOC], bf16, tag="scr")
                half = spool.tile([128, 4 * HALF], bf16, tag="half")
                exps.append((p2p, scr, half, ch0, nch))

            def do_group(gi, w):
                p2p, scr, half, ch0, nch = exps[gi]
                nc.scalar.activation(scr[:, :w], p2p[:, :w], AF.Exp)

            def do_reduce(gi):
                # bf16 pair-sum (DVE 2x fast path) then f32 reduce
                p2p, scr, half, ch0, nch = exps[gi]
                v = scr[:, :nch * NLOC].rearrange("p (a b) -> p a b",
                                                  b=NLOC)
                nc.vector.tensor_tensor(
                    half[:, :nch * HALF].rearrange("p (a b) -> p a b",
                                                   b=HALF),
                    v[:, :, 0:HALF], v[:, :, HALF:NLOC], op=OP.add)
                nc.vector.tensor_reduce(
                    stats[:, ch0:ch0 + nch],
                    half[:, :nch * HALF].rearrange("p (a b) -> p a b",
                                                   b=HALF),
                    axis=mybir.AxisListType.X, op=OP.add)

            # group 0's Exp covers chunks 0-2 plus the psi1 exponent
            do_group(0, 896)
            psi1c = exps[0][1][:, 768:896]        # bf16 scr slice
            for c in range(2):
                nc.tensor.matmul(apsum[:, :],
                                 lhsT=psi1c[:, c * 64:(c + 1) * 64],
                                 rhs=yb[:, c * 256:(c + 1) * 256],
                                 start=(c == 0), stop=(c == 1))
            do_reduce(0)
            nc.sync.dma_start(out=psi2_o[:, 0:3], in_=stats[:, 0:3])

            do_group(1, 1024)
            do_reduce(1)
            nc.vector.tensor_copy(a_sb[:, :], apsum[:, :])
            nc.sync.dma_start(out=psi2_o[:, 3:7], in_=stats[:, 3:7])
            nc.sync.dma_start(out=a_o[:, :], in_=a_sb[:, :])

            do_group(2, 1024)
            do_reduce(2)
            nc.sync.dma_start(out=psi2_o[:, 7:11], in_=stats[:, 7:11])

            do_group(3, 1024)
            do_reduce(3)
            nc.sync.dma_start(out=psi2_o[:, 11:15], in_=stats[:, 11:15])

            # last two chunks: n-sums fused into their Exps (accum_out)
            p2p4, scr4, _, ch04, _ = exps[4]
            for j in range(2):
                w0, w1 = j * NLOC, (j + 1) * NLOC
                nc.scalar.activation(scr4[:, w0:w1], p2p4[:, w0:w1],
                                     AF.Exp,
                                     accum_out=stats[:, ch04 + j:
                                                     ch04 + j + 1])
            nc.sync.dma_start(out=psi2_o[:, 15:17], in_=stats[:, 15:17])

    nc.compile()
    return nc


def _get_compiled():
    global _compiled
    if _compiled is None:
        _compiled = _build_bass()
    return _compiled


def kernel(y, q_mu, q_log_sigma, z, noise_raw, alpha, variance, _trace=False):
    import ml_dtypes
    from concourse.bass_utils import run_bass_kernel_spmd

    nc = _get_compiled()

    f8 = np.float64
    qm = q_mu.astype(f8)
    qls = q_log_sigma.astype(f8)
    z64 = z.astype(f8)
    al = alpha.astype(f8)
    var = f8(variance[0])
    logvar = np.log(var)

    # ---- host row prep (O(n*q)) ----
    qsig = np.logaddexp(qls, 0.0)                           # softplus
    d1 = qsig * al + 1.0
    d2 = 2.0 * al * qsig + 1.0
    w1 = al / d1
    w2 = al / d2
    lse1 = np.sum(np.log(d1), axis=1)                       # (n,)
    lse2 = np.sum(np.log(d2), axis=1)
    rt1 = np.sum(qm * qm * w1, axis=1)
    rt2 = np.sum(qm * qm * w2, axis=1)
    h1 = 2.0 * logvar - 0.5 * (rt1 + lse1)
    g = 4.0 * logvar - rt2 - 0.5 * lse2

    kl_sum = np.sum(-np.log(qsig) + 0.5 * (qsig * qsig + qm * qm - 1.0))
    tr_yy = np.sum(y.astype(f8) ** 2)

    # ---- z-side blocks (replicated); exp(-s1) applied on the host ----
    iu, ju = np.triu_indices(M)                             # (2080,)
    Su = z64[iu] + z64[ju]                                  # (2080, q)
    sqz = (z64[:, None, :] - z64[None, :, :]) ** 2          # (m, m, q)
    s1 = 0.25 * (sqz @ al)                                  # (m, m)
    zl = np.zeros((33, NCHUNK * 128), np.float32)
    zl[0:16, :NPAIRS] = Su.T
    zl[16:32, :NPAIRS] = (-0.25 * Su * Su).T
    zl[32, :NPAIRS] = 1.0

    zs = z64.T                                              # (q, m)

    in_maps = []
    for i in range(NCORES):
        sl = slice(i * NLOC, (i + 1) * NLOC)
        p2 = np.empty((33, NLOC), np.float32)
        p2[0:16] = (qm[sl] * w2[sl]).T
        p2[16:32] = w2[sl].T
        p2[32] = g[sl]

        za = np.zeros((33, 1600), np.float32)
        za[:, 0:256] = p2
        za[:, 256:1280] = zl[:, 0:1024]
        za[0:16, 1280:1536] = (qm[sl] * w1[sl]).T
        za[16:32, 1280:1536] = w1[sl].T
        za[32, 1280:1536] = h1[sl]
        za[0:16, 1536:1600] = zs
        za[16:32, 1536:1600] = -0.5 * zs * zs
        za[32, 1536:1600] = 1.0

        zb = np.zeros((33, 1408), np.float32)
        zb[:, 0:256] = p2
        zb[:, 256:1408] = zl[:, 1024:]

        yb = np.ascontiguousarray(
            y[sl].astype(ml_dtypes.bfloat16).reshape(2, 128, D)
            .transpose(1, 0, 2).reshape(128, 512))

        in_maps.append({
            "za": za.astype(ml_dtypes.bfloat16),
            "zb": zb.astype(ml_dtypes.bfloat16),
            "yb": yb,
        })

    br = run_bass_kernel_spmd(nc, in_maps, list(range(NCORES)), trace=_trace)
    res = br.results

    psi2_part = np.zeros((128, NCHUNK), f8)
    A = np.zeros((M, D), f8)
    for r in res:
        psi2_part += r["out_psi2"].astype(f8)
        A += r["out_A"].astype(f8)

    flat = psi2_part.T.reshape(NCHUNK * 128)
    psi2 = np.empty((M, M), f8)
    es1 = np.exp(-s1[iu, ju])
    psi2[iu, ju] = flat[:NPAIRS] * es1
    psi2[ju, iu] = psi2[iu, ju]

    kl_term = kl_sum / (N * D)

    # small m x m algebra on host
    k_mm = var * np.exp(-0.5 * (sqz @ al))                  # (m, m)
    noise_var = np.logaddexp(f8(noise_raw[0]), 0.0)
    beta = 1.0 / noise_var
    psi0 = N * var

    cov1 = beta * psi2 + k_mm
    B = np.linalg.solve(cov1, A)
    tr_yWy = beta * tr_yy - np.sum(A * B)

    F = 0.5 * N * np.log(beta)
    F += 0.5 * np.linalg.slogdet(k_mm)[1]
    F -= 0.5 * N * np.log(np.pi)
    F -= 0.5 * np.linalg.slogdet(cov1)[1]
    F -= 0.5 * beta * psi0
    F += 0.5 * np.trace(np.linalg.solve(k_mm, psi2))
    F = (F * D - 0.5 * tr_yWy) / (N * D)

    out = F - kl_term
    result = np.asarray(out, dtype=np.float32)
    if _trace:
        return result, br
    return result


# revision 28
# speedup vs baseline: 1.1452x; 1.1452x over previous
"""Bayesian GPLVM collapsed-ELBO kernel for Trainium2 (8 NeuronCores).

Sharding: data-parallel over n (2048 rows -> 256 per core). All O(n*q)
row prep (softplus, d1/d2, w1/w2, log-sums, KL pieces) is done on host
in float64 and shipped as small per-core bf16 operand blocks; the
device does only the O(n*m) / O(n*m^2) work:

  - psi2: 2080 upper-triangle (i,j) pairs in 128-partition chunks;
    exponent = zl_chunk^T @ p2 (33x128x256 bf16) on PE, Exp on ACT,
    free-axis n-sum on DVE (bf16 pair-sum fast path + f32 reduce).
    exp(-s1[ij]) is folded in on the host after gathering.
  - group 0's PSUM tile carries chunk 16 (the 32-pair remainder), 3
    full chunks AND the psi1 exponent, so a single ACTIVATE serves all
    of them; chunk 15's n-sum rides its Exp via accum_out. A = psi1^T y
    accumulates in PSUM and drains via a DVE copy + DMA.

DMA plan: transfer rate scales with SBUF partition count and suffers
under ~1KB per-partition lines, so all 33-partition operands are
packed into TWO wide transfers (za: p2 + zl chunks 0-7 + chunk 16 +
p1 + zs1 | zb: p2 dup + zl chunks 8-15) that land at partition bases
0 and 64 of one tile and ride two different HWDGE queues in parallel
(matmul lhsT/rhs must share a base partition, hence the p2 dup). Only
the Exp table is ever needed: one ACT_TABLE_LOAD, overlapped with DMA.

SBUF column maps (bf16), base-0 block [33 x 1632]:
  0:256 p2 | 256:1280 zl ch0-7 | 1280:1312 zl ch16 | 1312:1568 p1 |
  1568:1632 zs1
base-64 block [33 x 1280]:  0:256 p2 | 256:1280 zl ch8-15

Device stats columns: col 0 = chunk 16, col 1+k = chunk k (k=0..15).
"""

import numpy as np

N, D, Q, M = 2048, 256, 16, 64
NCORES = 8
NLOC = N // NCORES          # 256
NPAIRS = 2080               # upper-triangle pairs of 64x64
NCHUNK = 17
GROUPS = [(3, 4), (7, 4), (11, 4)]   # 4-chunk PSUM groups after group 0
HALF = 128

_compiled = None


def _build_bass():
    import concourse.bacc as bacc
    import concourse.bass as bass  # noqa: F401
    import concourse.mybir as mybir
    from concourse.tile import TileContext

    f32 = mybir.dt.float32
    bf16 = mybir.dt.bfloat16
    AF = mybir.ActivationFunctionType
    OP = mybir.AluOpType

    nc = bacc.Bacc("TRN2", target_bir_lowering=False, num_swdge_queues=2)

    za_d = nc.declare_dram_parameter("za", [33, 1632], bf16, isOutput=False)
    zb_d = nc.declare_dram_parameter("zb", [33, 1280], bf16, isOutput=False)
    y_d = nc.declare_dram_parameter("yb", [128, 512], bf16, isOutput=False)
    psi2_o = nc.declare_dram_parameter("out_psi2", [128, NCHUNK], f32,
                                       isOutput=True)
    a_o = nc.declare_dram_parameter("out_A", [M, D], f32, isOutput=True)

    with TileContext(nc) as tc:
        with (
            tc.tile_pool(name="const", bufs=1) as cpool,
            tc.tile_pool(name="scr", bufs=2) as spool,
            tc.tile_pool(name="psum0", bufs=1, space="PSUM") as ppool0,
            tc.tile_pool(name="psum", bufs=2, space="PSUM") as ppool,
            tc.tile_pool(name="psuma", bufs=1, space="PSUM") as ppool_a,
        ):
            zt = cpool.tile([97, 1632], bf16)
            yb = cpool.tile([128, 512], bf16)
            stats = cpool.tile([128, NCHUNK], f32)
            a_sb = cpool.tile([M, D], f32)

            # two partition-parallel packed input streams; the first
            # is split so the matmul train can start sooner
            nc.sync.dma_start(out=zt[0:33, 0:768], in_=za_d[:, 0:768])
            nc.sync.dma_start(out=zt[0:33, 768:1632], in_=za_d[:, 768:1632])
            nc.scalar.dma_start(out=zt[64:97, 0:1280], in_=zb_d[:, :])
            nc.scalar.dma_start(out=yb[:, :], in_=y_d[:, :])

            apsum = ppool_a.tile([M, D], f32)

            def zl_lhsT(ch):
                if ch < 8:
                    return zt[0:33, 256 + ch * 128:256 + (ch + 1) * 128]
                if ch == 16:
                    return zt[0:33, 1280:1312]
                return zt[64:97, 256 + (ch - 8) * 128:
                          256 + (ch - 7) * 128]

            def p2_rhs(ch):
                return zt[0:33, 0:256] if (ch < 8 or ch == 16) else \
                    zt[64:97, 0:256]

            # group 0: [ch16 | ch0 | ch1 | ch2 | psi1-exponent]
            g0 = ppool0.tile([128, 1152], f32)
            for j, ch in enumerate([16, 0, 1, 2]):
                nc.tensor.matmul(
                    g0[0:32 if ch == 16 else 128,
                       j * NLOC:(j + 1) * NLOC],
                    lhsT=zl_lhsT(ch), rhs=p2_rhs(ch),
                    start=True, stop=True)
            for c in range(2):
                nc.tensor.matmul(
                    g0[:, 1024 + c * 64:1024 + (c + 1) * 64],
                    lhsT=zt[0:33, 1312 + c * 128:1312 + (c + 1) * 128],
                    rhs=zt[0:33, 1568:1632],
                    start=True, stop=True)

            ptiles = []
            for gi, (ch0, nch) in enumerate(GROUPS):
                p2p = ppool.tile([128, 4 * NLOC], f32, tag="p2p")
                ptiles.append((p2p, ch0, nch))
                for j in range(nch):
                    ch = ch0 + j
                    nc.tensor.matmul(
                        p2p[:, j * NLOC:(j + 1) * NLOC],
                        lhsT=zl_lhsT(ch), rhs=p2_rhs(ch),
                        start=True, stop=True)
            g4 = ppool.tile([128, 4 * NLOC], f32, tag="p2p")
            nc.tensor.matmul(g4[:, 0:NLOC], lhsT=zl_lhsT(15),
                             rhs=p2_rhs(15), start=True, stop=True)

            scr0 = spool.tile([128, 1152], bf16, tag="scr")
            half0 = spool.tile([128, 4 * HALF], bf16, tag="half")

            def pair_reduce(scr, half, w, out_cols):
                v = scr[:, :w].rearrange("p (a b) -> p a b", b=NLOC)
                nc.vector.tensor_tensor(
                    half[:, :w // 2].rearrange("p (a b) -> p a b",
                                               b=HALF),
                    v[:, :, 0:HALF], v[:, :, HALF:NLOC], op=OP.add)
                nc.vector.tensor_reduce(
                    out_cols,
                    half[:, :w // 2].rearrange("p (a b) -> p a b",
                                               b=HALF),
                    axis=mybir.AxisListType.X, op=OP.add)

            # group 0 Exp covers [ch16, ch0-2, psi1]; A matmuls follow
            nc.scalar.activation(scr0[:, :], g0[:, :], AF.Exp)
            psi1c = scr0[:, 1024:1152]
            for c in range(2):
                nc.tensor.matmul(apsum[:, :],
                                 lhsT=psi1c[:, c * 64:(c + 1) * 64],
                                 rhs=yb[:, c * 256:(c + 1) * 256],
                                 start=(c == 0), stop=(c == 1))
            pair_reduce(scr0, half0, 1024, stats[:, 0:4])
            nc.sync.dma_start(out=psi2_o[:, 0:4], in_=stats[:, 0:4])

            for gi, (p2p, ch0, nch) in enumerate(ptiles):
                scr = spool.tile([128, 1152], bf16, tag="scr")
                half = spool.tile([128, 4 * HALF], bf16, tag="half")
                w = nch * NLOC
                nc.scalar.activation(scr[:, :w], p2p[:, :w], AF.Exp)
                if gi == 0:
                    nc.vector.tensor_copy(a_sb[:, :], apsum[:, :])
                pair_reduce(scr, half, w,
                            stats[:, ch0 + 1:ch0 + 1 + nch])
                if gi == 0:
                    nc.sync.dma_start(out=a_o[:, :], in_=a_sb[:, :])
                if gi < 2:
                    nc.sync.dma_start(
                        out=psi2_o[:, ch0 + 1:ch0 + 1 + nch],
                        in_=stats[:, ch0 + 1:ch0 + 1 + nch])

            # chunk 15: n-sum fused into its Exp (accum_out); final DMA
            # carries cols 12:17 once the last reduce lands
            scr4 = spool.tile([128, NLOC], bf16)
            nc.scalar.activation(scr4[:, :], g4[:, 0:NLOC], AF.Exp,
                                 accum_out=stats[:, 16:17])
            nc.sync.dma_start(out=psi2_o[:, 12:17], in_=stats[:, 12:17])

    nc.compile()
    return nc


def _get_compiled():
    global _compiled
    if _compiled is None:
        _compiled = _build_bass()
    return _compiled


def kernel(y, q_mu, q_log_sigma, z, noise_raw, alpha, variance, _trace=False):
    import ml_dtypes
    from concourse.bass_utils import run_bass_kernel_spmd

    nc = _get_compiled()

    f8 = np.float64
    qm = q_mu.astype(f8)
    qls = q_log_sigma.astype(f8)
    z64 = z.astype(f8)
    al = alpha.astype(f8)
    var = f8(variance[0])
    logvar = np.log(var)

    # ---- host row prep (O(n*q)) ----
    qsig = np.logaddexp(qls, 0.0)                           # softplus
    d1 = qsig * al + 1.0
    d2 = 2.0 * al * qsig + 1.0
    w1 = al / d1
    w2 = al / d2
    lse1 = np.sum(np.log(d1), axis=1)                       # (n,)
    lse2 = np.sum(np.log(d2), axis=1)
    rt1 = np.sum(qm * qm * w1, axis=1)
    rt2 = np.sum(qm * qm * w2, axis=1)
    h1 = 2.0 * logvar - 0.5 * (rt1 + lse1)
    g = 4.0 * logvar - rt2 - 0.5 * lse2

    kl_sum = np.sum(-np.log(qsig) + 0.5 * (qsig * qsig + qm * qm - 1.0))
    tr_yy = np.sum(y.astype(f8) ** 2)

    # ---- z-side blocks (replicated); exp(-s1) applied on the host ----
    iu, ju = np.triu_indices(M)                             # (2080,)
    Su = z64[iu] + z64[ju]                                  # (2080, q)
    sqz = (z64[:, None, :] - z64[None, :, :]) ** 2          # (m, m, q)
    s1 = 0.25 * (sqz @ al)                                  # (m, m)
    zl = np.zeros((33, NCHUNK * 128), np.float32)
    zl[0:16, :NPAIRS] = Su.T
    zl[16:32, :NPAIRS] = (-0.25 * Su * Su).T
    zl[32, :NPAIRS] = 1.0

    zs = z64.T                                              # (q, m)

    in_maps = []
    for i in range(NCORES):
        sl = slice(i * NLOC, (i + 1) * NLOC)
        p2 = np.empty((33, NLOC), np.float32)
        p2[0:16] = (qm[sl] * w2[sl]).T
        p2[16:32] = w2[sl].T
        p2[32] = g[sl]

        za = np.zeros((33, 1632), np.float32)
        za[:, 0:256] = p2
        za[:, 256:1280] = zl[:, 0:1024]
        za[:, 1280:1312] = zl[:, 2048:2080]
        za[0:16, 1312:1568] = (qm[sl] * w1[sl]).T
        za[16:32, 1312:1568] = w1[sl].T
        za[32, 1312:1568] = h1[sl]
        za[0:16, 1568:1632] = zs
        za[16:32, 1568:1632] = -0.5 * zs * zs
        za[32, 1568:1632] = 1.0

        zb = np.zeros((33, 1280), np.float32)
        zb[:, 0:256] = p2
        zb[:, 256:1280] = zl[:, 1024:2048]

        yb = np.ascontiguousarray(
            y[sl].astype(ml_dtypes.bfloat16).reshape(2, 128, D)
            .transpose(1, 0, 2).reshape(128, 512))

        in_maps.append({
            "za": za.astype(ml_dtypes.bfloat16),
            "zb": zb.astype(ml_dtypes.bfloat16),
            "yb": yb,
        })

    br = run_bass_kernel_spmd(nc, in_maps, list(range(NCORES)), trace=_trace)
    res = br.results

    psi2_part = np.zeros((128, NCHUNK), f8)
    A = np.zeros((M, D), f8)
    for r in res:
        psi2_part += r["out_psi2"].astype(f8)
        A += r["out_A"].astype(f8)

    # device cols: [ch16 | ch0..ch15] -> flat pair order
    dev = psi2_part.T.reshape(NCHUNK * 128)
    flat = np.concatenate([dev[128:], dev[0:32]])
    psi2 = np.empty((M, M), f8)
    es1 = np.exp(-s1[iu, ju])
    psi2[iu, ju] = flat[:NPAIRS] * es1
    psi2[ju, iu] = psi2[iu, ju]

    kl_term = kl_sum / (N * D)

    # small m x m algebra on host
    k_mm = var * np.exp(-0.5 * (sqz @ al))                  # (m, m)
    noise_var = np.logaddexp(f8(noise_raw[0]), 0.0)
    beta = 1.0 / noise_var
    psi0 = N * var

    cov1 = beta * psi2 + k_mm
    B = np.linalg.solve(cov1, A)
    tr_yWy = beta * tr_yy - np.sum(A * B)

    F = 0.5 * N * np.log(beta)
    F += 0.5 * np.linalg.slogdet(k_mm)[1]
    F -= 0.5 * N * np.log(np.pi)
    F -= 0.5 * np.linalg.slogdet(cov1)[1]
    F -= 0.5 * beta * psi0
    F += 0.5 * np.trace(np.linalg.solve(k_mm, psi2))
    F = (F * D - 0.5 * tr_yWy) / (N * D)

    out = F - kl_term
    result = np.asarray(out, dtype=np.float32)
    if _trace:
        return result, br
    return result
